# revision 27
# baseline (speedup 1.0000x reference)
"""Trainium2 Bass kernel for nn_HandIntersectionLoss.

Strategy
--------
Pure data parallel over batch: 64 batches -> 8 cores x 8 local batches.

Wall-clock per call is dominated by the axon tunnel, so the host ships
only the gathered hand points (~140KB/core) and the device derives all
per-(batch,face) matmul constants itself:

  phase 0 (device):
    - one-hot face matrices from f32 face indices (K=1 broadcast matmul
      + is_equal against shipped iota columns)
    - triangle corners A,B,C per (batch,dir) via 2-chunk accumulated
      gather matmuls:  corners[3,500] = pts[128,3]^T @ onehot[128,500]
    - edges E1=B-A, E2=C-A; normal n = E1 x E2 via permutation-matmul
      rotations (engines cannot read partition offsets != 0)
    - dots |A|^2,.., 2A.B,.., 2A.n via ones/twos-column reduce matmuls
    - constants assembled into a persistent `staged` SBUF tile
      ([65,7,512]: 4 rows per (batch,dir) + shared coefficient row)
      via SBUF->SBUF DMAs (the only legal cross-partition mover)

  phase 1 (device): the proven compute loop. Per 128-point block:
    K=5 matmuls against staged constants produce la^2,lb^2,lc^2,
    2ab,2bc,2ca, 2det for [128 points x 500 faces]; per-element chain
    (denominator + range-reduced atan2) on DVE/ACT:

      atan2(det, den) = 2*atan(det / (rho + |den|))            (den >= 0)
                      = sign(det)*pi - 2*atan(det/(rho+|den|)) (den < 0)
      rho = sqrt(det^2 + den^2 + 1e-20)

    inside(p) <=> sum_f half > pi/2.  Min-distance via the same matmul
    trick against derived vert constants (mrhs) + free-dim min-reduce.
    Scalar-engine table sets force the two-pass structure (sqrt vs
    arctan live in different ACT table sets), staged in super-groups.

The jitted shard_map callable is cached across kernel() calls so repeat
calls skip jax retrace/XLA recompile entirely.

Group semantics (raw, no halving on device):
  g0..2: xyz=A|B|C,       c3=|A|^2..,  w=1
  g3..5: xyz=(A+B)..raw,  c3=2A.B..,   w=2   -> col = 2*(A-p).(B-p)
  g6:    xyz=n raw,       c3=2*A.n,    w=0   -> col = 2*det
pass_a compensates with x0.5 folded into existing scalar_tensor_tensor.
"""
import os
import sys
import numpy as np

sys.path.insert(0, '/opt/trn_rl_repo')

B, V_FULL, V_HAND, V_LOOP, N_FACES = 64, 6890, 250, 20, 500
P = V_HAND + 1          # 251 points/verts per hand (incl. lid)
PPAD = 256
NCORES = 8
NB = B // NCORES        # local batches per core
NBD = NB * 2            # (batch, dir) pairs per core
NBLK = NBD * 2          # blocks per core: x2 point-chunks of 128
SUPER = 8               # blocks per two-pass super-group
F = N_FACES
HALF_PI = float(np.pi / 2)

_compiled = None
SKIP_P1 = False
_runner = None
last_exec_time_ns = None


# --------------------------------------------------------------------------
# host prep: index gathers only (all heavy constant math moved on-device)
# --------------------------------------------------------------------------

# preallocated per-call buffers (pad columns written once; concat layouts
# built directly to skip per-core copies)
_pts_host = np.full((B, 2, PPAD, 3), 1e2, np.float32)
_pts_concat = np.empty((NCORES * 128, 2, NBD, 3), np.float32)
_faces_concat = np.full((NCORES, 2, 3, 512), 300.0, np.float16)
_cst_concat = np.zeros((NCORES * 128, 8), np.float32)
for _c in range(NCORES):
    _cs = _cst_concat[_c * 128:(_c + 1) * 128]
    _cs[:, 0] = np.arange(128, dtype=np.float32)
    _cs[:, 1] = np.arange(128, 256, dtype=np.float32)
    for _m in range(3):
        _cs[(_m + 1) % 3, 2 + _m] = 1.0    # P1 (rot1)
        _cs[(_m + 2) % 3, 5 + _m] = 1.0    # P2 (rot2)
_extra_concat = np.ascontiguousarray(
    np.broadcast_to(np.arange(PPAD, dtype=np.float32), (NCORES, PPAD)))


def _prep_pts(inputs):
    verts = np.asarray(inputs['verts_batch'], dtype=np.float32)
    hi = [np.asarray(inputs['hand_verts_inds_left']),
          np.asarray(inputs['hand_verts_inds_right'])]
    li = [np.asarray(inputs['hand_loop_verts_inds_left']),
          np.asarray(inputs['hand_loop_verts_inds_right'])]

    # pad stays 1e2 from init (pad cols never overwritten)
    for d in range(2):
        _pts_host[:, d, :V_HAND] = verts[:, hi[d]]
        _pts_host[:, d, V_HAND] = verts[:, li[d]].mean(axis=1,
                                                       dtype=np.float32)

    # [core*128, 2kk, bd, 3] gather layout in one strided copy
    _pts_concat.reshape(NCORES, 128, 2, NBD, 3)[:] = \
        _pts_host.reshape(NCORES, NBD, 2, 128, 3).transpose(0, 3, 2, 1, 4)
    return _pts_concat


def _prep_faces(inputs):
    fc = [np.asarray(inputs['hand_faces_left']),
          np.asarray(inputs['hand_faces_right'])]
    for s in range(2):
        _faces_concat[:, s, :, :F] = fc[s].T.astype(np.float16)[None]
    return _faces_concat


# --------------------------------------------------------------------------
# device kernel
# --------------------------------------------------------------------------

def _kernel_body(tc, pts_d, faces_d, cst_d, extra_d, loss_d):
    import concourse.mybir as mybir
    nc = tc.nc
    fp32 = mybir.dt.float32
    AF = mybir.ActivationFunctionType
    OP = mybir.AluOpType
    AX = mybir.AxisListType.X

    fp16 = mybir.dt.float16
    with tc.tile_pool(name="const", bufs=1) as cpool:
        lhsT_sb = cpool.tile([5, NBD, PPAD], fp32)
        mrhs_sb = cpool.tile([5, NBD, PPAD], fp32)
        staged = cpool.tile([80, 7, 512], fp32)
        ones = cpool.tile([128, 1], fp32)
        beps = cpool.tile([128, 1], fp32)
        sacc = cpool.tile([128, NBLK], fp32)
        minda = cpool.tile([128, NBLK], fp32)
        nc.vector.memset(ones[:], 1.0)
        nc.vector.memset(beps[:], 1e-12)

        # ---------------- phase 0: derive constants on device ----------
        with tc.tile_pool(name="ph0", bufs=1) as zp:
            ones1 = zp.tile([1, 128], fp32)
            ones1h = zp.tile([1, 128], fp16)
            ones3 = zp.tile([3, 1], fp32)
            twos3 = zp.tile([3, 1], fp32)
            nc.vector.memset(ones1[:], 1.0)
            nc.vector.memset(ones1h[:], 1.0)
            nc.vector.memset(ones3[:], 1.0)
            nc.vector.memset(twos3[:], 2.0)
            pts_sb = zp.tile([128, 2, NBD, 3], fp32)
            faces_sb = zp.tile([1, 2, 3, 512], fp16)
            cst_sb = zp.tile([128, 8], fp32)
            extra_sb = zp.tile([1, PPAD], fp32)
            nc.sync.dma_start(pts_sb[:], pts_d[:])
            nc.sync.dma_start(faces_sb[:], faces_d[:])
            nc.sync.dma_start(cst_sb[:], cst_d[:])
            nc.sync.dma_start(extra_sb[:], extra_d[:])

            # shared coefficient row (DMA'd into each bd's staged block)
            rc = zp.tile([1, 7, 512], fp32)
            nc.vector.memset(rc[:, 0:3, :], 1.0)
            nc.vector.memset(rc[:, 3:6, :], 2.0)
            nc.vector.memset(rc[:, 6:7, :], 0.0)

            # one-hot face matrices per hand s, corner k, K-chunk kk
            # + identity one-hot (for pts transposition via gather matmul)
            oh = zp.tile([128, 2, 3, 2, 512], fp32)
            idh = zp.tile([128, 2, PPAD], fp32)
            PT = zp.tile([3, PPAD], fp32)
            SQ = zp.tile([3, PPAD], fp32)
            one256 = zp.tile([1, PPAD], fp32)
            nc.vector.memset(one256[:], 1.0)
            with tc.tile_pool(name="ph0bc", bufs=1, space="PSUM") as bp:
                bc = bp.tile([128, 3, 512], fp32)
                bcid = bp.tile([128, PPAD], fp32)
                ptp = bp.tile([3, PPAD], fp32)
                sqp = bp.tile([1, PPAD], fp32)
                for s in range(2):
                    for k in range(3):
                        nc.tensor.matmul(bc[:, k, :], ones1h[:],
                                         faces_sb[:, s, k, :])
                    for k in range(3):
                        for kk in range(2):
                            nc.vector.tensor_scalar(
                                oh[:, s, k, kk, :], bc[:, k, :],
                                cst_sb[:, kk:kk + 1], None, OP.is_equal)
                nc.tensor.matmul(bcid[:], ones1[:], extra_sb[:])
                for kk in range(2):
                    nc.vector.tensor_scalar(idh[:, kk, :], bcid[:],
                                            cst_sb[:, kk:kk + 1], None,
                                            OP.is_equal)
                # lhsT rows from pts: -2*pts^T via identity-gather matmuls,
                # |p|^2 via square + ones3-reduce
                for bd in range(NBD):
                    nc.tensor.matmul(ptp[:], pts_sb[:, 0, bd, :],
                                     idh[:, 0, :], start=True, stop=False)
                    nc.tensor.matmul(ptp[:], pts_sb[:, 1, bd, :],
                                     idh[:, 1, :], start=False, stop=True)
                    nc.scalar.mul(lhsT_sb[0:3, bd, :], ptp[:], -2.0)
                    nc.scalar.activation(PT[:], ptp[:], AF.Copy)
                    nc.vector.tensor_tensor(SQ[:], PT[:], PT[:], OP.mult)
                    nc.tensor.matmul(sqp[:], ones3[:], SQ[:])
                    sq1 = zp.tile([1, PPAD], fp32, name="sq1", tag="sq1",
                                  bufs=2)
                    nc.scalar.activation(sq1[:], sqp[:], AF.Copy)
                    nc.sync.dma_start(lhsT_sb[4:5, bd, :], sq1[:])
                    nc.sync.dma_start(lhsT_sb[3:4, bd, :], one256[:])

            # mrhs: rows0..2 = -0.5*lhsT rows0..2 (= vert xyz),
            # row3 <- lhsT row4 (|v|^2), row4 <- lhsT row3 (ones)
            nc.vector.tensor_scalar(mrhs_sb[0:3], lhsT_sb[0:3], -0.5, None,
                                    OP.mult)
            nc.sync.dma_start(mrhs_sb[3:4], lhsT_sb[4:5])
            nc.sync.dma_start(mrhs_sb[4:5], lhsT_sb[3:4])

            E1 = zp.tile([3, 512], fp32)
            E2 = zp.tile([3, 512], fp32)
            rotc = zp.tile([3, 4, 512], fp32)
            t1 = zp.tile([3, 512], fp32)
            t2 = zp.tile([3, 512], fp32)

            with tc.tile_pool(name="ph0ps", bufs=1, space="PSUM") as pp0:
                crn = [pp0.tile([3, 512], fp32, name=f"crn{t}", tag=t)
                       for t in "abc"]
                rot = pp0.tile([3, 4, 512], fp32)
                c3p = pp0.tile([1, 512], fp32)
                for bd in range(NBD):
                    other = bd ^ 1
                    s = other & 1
                    for k in range(3):
                        nc.tensor.matmul(crn[k][:], pts_sb[:, 0, other, :],
                                         oh[:, s, k, 0, :],
                                         start=True, stop=False)
                        nc.tensor.matmul(crn[k][:], pts_sb[:, 1, other, :],
                                         oh[:, s, k, 1, :],
                                         start=False, stop=True)
                    # double-buffered so bd+1's build overlaps bd's tail
                    asm = zp.tile([3, 7, 512], fp32, name="asm", tag="asm",
                                  bufs=2)
                    PRD = zp.tile([3, 7, 512], fp32, name="PRD", tag="PRD",
                                  bufs=2)
                    C3r = zp.tile([1, 7, 512], fp32, name="C3r", tag="C3r",
                                  bufs=2)
                    Ac, Bc, Cc = (asm[:, 0, :], asm[:, 1, :], asm[:, 2, :])
                    nc.scalar.activation(Ac, crn[0][:], AF.Copy)
                    nc.scalar.activation(Bc, crn[1][:], AF.Copy)
                    nc.scalar.activation(Cc, crn[2][:], AF.Copy)
                    nc.vector.tensor_tensor(E1[:], Bc, Ac, OP.subtract)
                    nc.vector.tensor_tensor(E2[:], Cc, Ac, OP.subtract)
                    # n = E1 x E2 via rotations: rot1/rot2 = P1^T/P2^T
                    nc.tensor.matmul(rot[:, 0, :], cst_sb[0:3, 2:5], E1[:])
                    nc.tensor.matmul(rot[:, 1, :], cst_sb[0:3, 5:8], E2[:])
                    nc.tensor.matmul(rot[:, 2, :], cst_sb[0:3, 5:8], E1[:])
                    nc.tensor.matmul(rot[:, 3, :], cst_sb[0:3, 2:5], E2[:])
                    nc.scalar.activation(rotc[:], rot[:], AF.Copy)
                    nc.vector.tensor_tensor(t1[:], rotc[:, 0, :],
                                            rotc[:, 1, :], OP.mult)
                    nc.vector.tensor_tensor(t2[:], rotc[:, 2, :],
                                            rotc[:, 3, :], OP.mult)
                    nc.vector.tensor_tensor(asm[:, 6, :], t1[:], t2[:],
                                            OP.subtract)
                    # products for the c3 reduces + midpoint sums
                    nc.vector.tensor_tensor(PRD[:, 0, :], Ac, Ac, OP.mult)
                    nc.vector.tensor_tensor(PRD[:, 1, :], Bc, Bc, OP.mult)
                    nc.vector.tensor_tensor(PRD[:, 2, :], Cc, Cc, OP.mult)
                    nc.vector.tensor_tensor(PRD[:, 3, :], Ac, Bc, OP.mult)
                    nc.vector.tensor_tensor(PRD[:, 4, :], Bc, Cc, OP.mult)
                    nc.vector.tensor_tensor(PRD[:, 5, :], Cc, Ac, OP.mult)
                    nc.vector.tensor_tensor(PRD[:, 6, :], Ac, asm[:, 6, :],
                                            OP.mult)
                    nc.vector.tensor_tensor(asm[:, 3, :], Ac, Bc, OP.add)
                    nc.vector.tensor_tensor(asm[:, 4, :], Bc, Cc, OP.add)
                    nc.vector.tensor_tensor(asm[:, 5, :], Cc, Ac, OP.add)
                    for g in range(7):
                        nc.tensor.matmul(c3p[:], ones3[:] if g < 3 else twos3[:],
                                         PRD[:, g, :])
                        nc.scalar.activation(C3r[:, g, :], c3p[:], AF.Copy)
                    # assemble this bd's staged block: xyz rows, c3 row, coeffs
                    nc.sync.dma_start(staged[5 * bd:5 * bd + 3], asm[:])
                    nc.sync.dma_start(staged[5 * bd + 3:5 * bd + 4], C3r[:])
                    nc.sync.dma_start(staged[5 * bd + 4:5 * bd + 5], rc[:])

        # ---------------- phase 1: main compute loop --------------------
        with (
            tc.tile_pool(name="store", bufs=1) as spool,
            tc.tile_pool(name="stage", bufs=2) as stpool,
            tc.tile_pool(name="iface", bufs=2) as ipool,
            tc.tile_pool(name="dve", bufs=1) as vpool,
        ):
            denoms = spool.tile([128, SUPER, 512], fp32)
            tts = spool.tile([128, SUPER, 512], fp32)

            def pass_a(ppool, i, j):
                bd, ch = divmod(i, 2)
                if ch == 0:
                    fstage = stpool.tile([5, 7, 512], fp32, tag="fstage")
                    nc.sync.dma_start(fstage[:], staged[5 * bd:5 * bd + 5])
                    pass_a.stage = fstage
                fstage = pass_a.stage
                lhs = lhsT_sb[:, bd, ch * 128:(ch + 1) * 128]

                wind = ppool.tile([128, 7, 512], fp32, tag="wind")
                md = ppool.tile([128, 256], fp32, tag="md")

                for g in range(7):
                    nc.tensor.matmul(wind[:, g, :F], lhs, fstage[:, g, :F])
                nc.tensor.matmul(md[:, :P], lhs, mrhs_sb[:, bd ^ 1, :P])

                # min-distance: free-dim min, clamp at 0 (matmul roundoff)
                mind = vpool.tile([128, 1], fp32, tag="mind")
                nc.vector.tensor_reduce(mind[:], md[:, :P], AX, OP.min)
                nc.vector.tensor_scalar(minda[:, i:i + 1], mind[:], 0.0, None,
                                        OP.max)

                # norms: clamp squared lengths at 0, sqrt (one multi-dim-AP
                # op per stage instead of three)
                rl = ipool.tile([128, 3, 512], fp32, tag="rl")
                nc.scalar.activation(rl[:, :, :F], wind[:, 0:3, :F], AF.Relu)
                sq3 = ipool.tile([128, 3, 512], fp32, tag="sq3")
                nc.scalar.activation(sq3[:, :, :F], rl[:, :, :F], AF.Sqrt)
                la = sq3[:, 0, :]
                lb = sq3[:, 1, :]
                lc = sq3[:, 2, :]
                dets = ipool.tile([128, 512], fp32, tag="dets")
                nc.scalar.mul(dets[:, :F], wind[:, 6, :F], 0.5)

                # denominator chain; wind groups 3..5 hold 2ab/2bc/2ca so
                # fold the x0.5 into the scalar_tensor_tensor ops
                u = vpool.tile([128, 512], fp32, tag="u")
                r4 = vpool.tile([128, 512], fp32, tag="r4")
                s5 = vpool.tile([128, 512], fp32, tag="s5")
                v = vpool.tile([128, 512], fp32, tag="v")
                w = vpool.tile([128, 512], fp32, tag="w")
                t6 = vpool.tile([128, 512], fp32, tag="t6")
                nc.vector.scalar_tensor_tensor(r4[:, :F], wind[:, 4, :F], 0.5,
                                               la[:, :F], OP.mult, OP.mult)
                nc.vector.scalar_tensor_tensor(s5[:, :F], wind[:, 5, :F], 0.5,
                                               lb[:, :F], OP.mult, OP.mult)
                nc.vector.tensor_tensor(u[:, :F], la[:, :F], lb[:, :F], OP.mult)
                nc.vector.scalar_tensor_tensor(v[:, :F], wind[:, 3, :F], 0.5,
                                               u[:, :F], OP.mult, OP.add)

                w_ = w[:, :F]
                nc.vector.tensor_tensor(w_, v[:, :F], lc[:, :F], OP.mult)
                nc.vector.tensor_tensor(t6[:, :F], r4[:, :F], s5[:, :F], OP.add)
                den = denoms[:, j, :F]
                nc.vector.tensor_tensor(den, w_, t6[:, :F], OP.add)

                # half-angle atan2 range reduction: tt = det / (rho + |den|)
                xx = ipool.tile([128, 512], fp32, tag="xx")
                yy = ipool.tile([128, 512], fp32, tag="yy")
                ss = vpool.tile([128, 512], fp32, tag="ss", bufs=2)
                rho = ipool.tile([128, 512], fp32, tag="rho")
                axd = ipool.tile([128, 512], fp32, tag="axd")
                dd = vpool.tile([128, 512], fp32, tag="dd")
                rd = vpool.tile([128, 512], fp32, tag="rd")
                nc.scalar.activation(xx[:, :F], den, AF.Square)
                nc.scalar.activation(yy[:, :F], dets[:, :F], AF.Square)
                nc.vector.scalar_tensor_tensor(ss[:, :F], xx[:, :F], 1e-20,
                                               yy[:, :F], OP.add, OP.add)
                nc.scalar.activation(rho[:, :F], ss[:, :F], AF.Sqrt)
                nc.scalar.activation(axd[:, :F], den, AF.Abs)
                nc.vector.tensor_tensor(dd[:, :F], rho[:, :F], axd[:, :F],
                                        OP.add)
                nc.vector.reciprocal_approx_fast(rd[:, :F], dd[:, :F])
                nc.vector.tensor_tensor(tts[:, j, :F], dets[:, :F], rd[:, :F],
                                        OP.mult)

            def pass_b(i, j):
                den = denoms[:, j, :F]
                tt = tts[:, j, :F]
                sgn = ipool.tile([128, 512], fp32, tag="sgn")
                spi = ipool.tile([128, 512], fp32, tag="spi")
                atn = ipool.tile([128, 512], fp32, tag="atn")
                c0 = vpool.tile([128, 512], fp32, tag="c0")
                c1 = vpool.tile([128, 512], fp32, tag="c1")
                sd = vpool.tile([128, 512], fp32, tag="sd")
                nc.scalar.activation(sgn[:, :F], tt, AF.Sign)
                nc.scalar.mul(spi[:, :F], sgn[:, :F], HALF_PI)
                nc.scalar.activation(atn[:, :F], tt, AF.Arctan)
                # half = atn + [den<0]*(pi/2*sign(det) - 2*atn)
                nc.vector.scalar_tensor_tensor(c0[:, :F], atn[:, :F], -2.0,
                                               spi[:, :F], OP.mult, OP.add)
                nc.vector.scalar_tensor_tensor(c1[:, :F], den, 0.0,
                                               c0[:, :F], OP.is_lt, OP.mult)
                nc.vector.scalar_tensor_tensor(sd[:, :F], atn[:, :F], 0.0,
                                               c1[:, :F], OP.add, OP.add,
                                               accum_out=sacc[:, i:i + 1])

            nc.vector.memset(sacc[:], 0.0)
            nc.vector.memset(minda[:], 1.0)
            with tc.tile_pool(name="psum", bufs=1, space="PSUM") as ppool:
                for sg in range(0 if SKIP_P1 else NBLK // SUPER):
                    for j in range(SUPER):
                        pass_a(ppool, sg * SUPER + j, j)
                    tc.no_sync_barrier()
                    for j in range(SUPER):
                        pass_b(sg * SUPER + j, j)
                    tc.no_sync_barrier()

            # ------------- final: depth * inside, partition-reduce -------
            inside = cpool.tile([128, NBLK], fp32)
            depth = cpool.tile([128, NBLK], fp32)
            contrib = cpool.tile([128, NBLK], fp32)
            nc.vector.tensor_scalar(inside[:], sacc[:], HALF_PI, None,
                                    OP.is_gt)
            nc.scalar.activation(depth[:], minda[:], AF.Sqrt, bias=beps[:])
            nc.vector.tensor_tensor(contrib[:], depth[:], inside[:], OP.mult)

            with tc.tile_pool(name="psum2", bufs=1, space="PSUM") as p2:
                lpsum = p2.tile([NBLK, 1], fp32)
                nc.tensor.matmul(lpsum[:], contrib[:], ones[:])
                loss_sb = cpool.tile([NBLK, 1], fp32)
                nc.scalar.activation(loss_sb[:], lpsum[:], AF.Copy)
                nc.sync.dma_start(loss_d[:], loss_sb[:])


def _build():
    global _compiled
    if _compiled is not None:
        return _compiled
    import concourse.bacc as bacc
    import concourse.mybir as mybir
    import concourse.tile as tile

    nc = bacc.Bacc("TRN2", target_bir_lowering=False, debug=False,
                   num_devices=NCORES)
    fp32 = mybir.dt.float32
    fp16 = mybir.dt.float16
    pts_d = nc.dram_tensor("pts", (128, 2, NBD, 3), fp32, kind="ExternalInput").ap()
    faces_d = nc.dram_tensor("faces", (1, 2, 3, 512), fp16, kind="ExternalInput").ap()
    cst_d = nc.dram_tensor("cst", (128, 8), fp32, kind="ExternalInput").ap()
    extra_d = nc.dram_tensor("extra", (1, PPAD), fp32, kind="ExternalInput").ap()
    loss_d = nc.dram_tensor("loss", (NBLK, 1), fp32, kind="ExternalOutput").ap()

    with tile.TileContext(nc) as tc:
        _kernel_body(tc, pts_d, faces_d, cst_d, extra_d, loss_d)
    nc.compile()
    _compiled = nc
    return nc


# --------------------------------------------------------------------------
# cached jitted runner + entry point
# --------------------------------------------------------------------------

def _build_runner():
    global _runner
    if _runner is not None:
        return _runner
    import jax
    from jax.sharding import Mesh, PartitionSpec
    from jax.experimental.shard_map import shard_map
    import concourse.mybir as mybir
    from concourse.bass2jax import (_bass_exec_p, partition_id_tensor,
                                    install_neuronx_cc_hook)

    nc = _build()
    install_neuronx_cc_hook()
    pname = nc.partition_id_tensor.name if nc.partition_id_tensor else None
    in_names, out_names, out_avals, zero_outs = [], [], [], []
    for alloc in nc.m.functions[0].allocations:
        if not isinstance(alloc, mybir.MemoryLocationSet):
            continue
        name = alloc.memorylocations[0].name
        if alloc.kind == "ExternalInput":
            if name != pname:
                in_names.append(name)
        elif alloc.kind == "ExternalOutput":
            out_names.append(name)
            shape = tuple(alloc.tensor_shape)
            dtype = mybir.dt.np(alloc.dtype)
            out_avals.append(jax.core.ShapedArray(shape, dtype))
            zero_outs.append(np.zeros(shape, dtype))
    n_params, n_outs = len(in_names), len(out_avals)
    in_names_full = in_names + out_names + ([pname] if pname else [])

    def _body(*args):
        operands = list(args)
        if pname is not None:
            operands.append(partition_id_tensor())
        return tuple(_bass_exec_p.bind(
            *operands, out_avals=tuple(out_avals), in_names=tuple(in_names_full),
            out_names=tuple(out_names), lowering_input_output_aliases=(),
            sim_require_finite=True, sim_require_nnan=True, nc=nc))

    devices = jax.devices()[:NCORES]
    mesh = Mesh(np.asarray(devices), ("core",))
    in_specs = (PartitionSpec("core"),) * (n_params + n_outs)
    out_specs = (PartitionSpec("core"),) * len(out_names)
    sharded = jax.jit(
        shard_map(_body, mesh=mesh, in_specs=in_specs, out_specs=out_specs,
                  check_rep=False),
        donate_argnums=tuple(range(n_params, n_params + n_outs)),
        keep_unused=True)
    czero_shapes = [((NCORES * z.shape[0],) + z.shape[1:], z.dtype)
                    for z in zero_outs]
    # cst/extra are pure algorithm constants (iota columns, permutation
    # matrices): commit them device-resident once; passing the committed
    # arrays skips their per-call host->device processing (~4ms).
    from jax.sharding import NamedSharding
    shc = NamedSharding(mesh, PartitionSpec("core"))
    resident = {"cst": jax.device_put(_cst_concat, shc),
                "extra": jax.device_put(_extra_concat, shc)}
    jax.block_until_ready(list(resident.values()))
    _runner = (sharded, in_names, czero_shapes, resident, shc)
    return _runner


def kernel(**inputs) -> np.ndarray:
    global last_exec_time_ns
    import jax
    sharded, in_names, czero_shapes, resident, shc = _build_runner()
    ptsdev = jax.device_put(_prep_pts(inputs), shc)   # transfer starts now
    facesdev = jax.device_put(_prep_faces(inputs), shc)
    by_name = {"pts": ptsdev, "faces": facesdev}
    concat_in = [resident.get(nm, by_name.get(nm)) for nm in in_names]
    zeros = [np.zeros(s, d) for s, d in czero_shapes]
    out = sharded(*concat_in, *zeros)
    last_exec_time_ns = None

    o0 = np.asarray(out[0]).reshape(NCORES, NBLK)
    loss = np.zeros(B, np.float32)
    for c in range(NCORES):
        # block i = (b_loc*2 + dir)*2 + chunk
        loss[c * NB:(c + 1) * NB] = o0[c].reshape(NB, 4).sum(axis=1)
    return loss
